# Initial kernel scaffold
#
"""Single-head attention (B=8, S=2048, D=1024) on 8 TRN2 NeuronCores.

Data-parallel over batch: core b handles batch element b entirely.

Per-core dataflow (all matmuls fp32r):
  Phase 1: K^T = (Wk x^T)  -> SBUF resident  [e, key]  (x^T resident)
  Phase 2: Q^T = (Wq x^T)  -> DRAM scratch   [qc, ec, p, q]
  Phase 3: V   = (x Wv^T)  -> SBUF resident  [key, e]  (x^T streamed as stationary)
  Phase B (per 128-query chunk):
    scores[q, key] = Q^T.T @ K^T  (+ maskbias via K=1 matmul)   [PSUM]
    E = exp(scores/32) via ACT (fp32r) with accum_out -> denom
    P^T tiles via PE transpose; PV: out = P^T.T @ V; out /= denom (DVE)

Softmax shift-invariance: reference subtracts rowmax; we subtract nothing
(scores are O(10), exp is safe in fp32) — mathematically identical.
Masked keys get scores-1e6 -> exp underflows to exactly 0.
bq/bk applied via ACT bias if nonzero; bv added on host (softmax rows sum to 1).
"""

import sys

sys.path.insert(0, "/opt/trn_rl_repo")

import numpy as np

import concourse.bacc as bacc
import concourse.tile as tile
from concourse import mybir
from concourse.bass_utils import run_bass_kernel_spmd

FP32R = mybir.dt.float32r
FP32 = mybir.dt.float32

S = 2048
D = 1024
NCORES = 8
NQC = S // 128      # 16 query/key chunks of 128
NKB = S // 512      # 4 key blocks of 512
NEC = D // 128      # 8 e-chunks
NDC = D // 128      # 8 d-chunks (contraction)
SCALE = 1.0 / np.sqrt(np.float32(D))


def build_nc(has_bq: bool, has_bk: bool, repeat: int = 1):
    nc = bacc.Bacc("TRN2", target_bir_lowering=False)

    XT = nc.dram_tensor("XT", [D, S], FP32R, kind="ExternalInput")
    WQT = nc.dram_tensor("WQT", [D, D], FP32R, kind="ExternalInput")
    WKT = nc.dram_tensor("WKT", [D, D], FP32R, kind="ExternalInput")
    WVT = nc.dram_tensor("WVT", [D, D], FP32R, kind="ExternalInput")
    MB = nc.dram_tensor("MB", [1, S], FP32R, kind="ExternalInput")
    ONES = nc.dram_tensor("ONES", [1, 128], FP32R, kind="ExternalInput")
    IDN = nc.dram_tensor("IDN", [128, 128], FP32R, kind="ExternalInput")
    BQ = nc.dram_tensor("BQ", [128, NEC], FP32, kind="ExternalInput")
    BK = nc.dram_tensor("BK", [128, NEC], FP32, kind="ExternalInput")
    OUT = nc.dram_tensor("OUT", [S, D], FP32, kind="ExternalOutput")

    Copy = mybir.ActivationFunctionType.Copy
    Exp = mybir.ActivationFunctionType.Exp

    with tile.TileContext(nc) as tc:
        with (
            tc.tile_pool(name="dram", bufs=1, space="DRAM") as dramp,
            tc.tile_pool(name="const", bufs=1) as constp,
            tc.tile_pool(name="ktp", bufs=1) as ktp,
        ):
            qt_dram = dramp.tile([NQC, NEC, 128, 128], FP32R)

            ident = constp.tile([128, 128], FP32R)
            ones = constp.tile([1, 128], FP32R)
            maskb = constp.tile([1, S], FP32R)
            nc.sync.dma_start(ident, IDN[:, :])
            nc.sync.dma_start(ones, ONES[:, :])
            nc.sync.dma_start(maskb, MB[:, :])
            bq_sb = constp.tile([128, NEC], FP32)
            bk_sb = constp.tile([128, NEC], FP32)
            if has_bq:
                nc.sync.dma_start(bq_sb, BQ[:, :])
            if has_bk:
                nc.sync.dma_start(bk_sb, BK[:, :])

            kt = ktp.tile([128, NEC, S], FP32R)  # K^T: [e%128, e//128, key]

            for rep_i in range(repeat):
                # ---- Phases 1+2: K^T and Q^T with x^T resident ----
                with (
                    tc.tile_pool(name="xtp", bufs=1) as xtp,
                    tc.tile_pool(name="wp", bufs=1) as wp,
                    tc.tile_pool(name="qsp", bufs=2) as qsp,
                    tc.tile_pool(name="psA", bufs=2, space="PSUM") as psA,
                ):
                    xt = xtp.tile([128, NDC, S], FP32R)
                    for d in range(NDC):
                        nc.sync.dma_start(
                            xt[:, d, :], XT[d * 128:(d + 1) * 128, :])

                    for phase in ("k", "q"):
                        w_sb = wp.tile([128, NDC, D], FP32R, tag="w")
                        WT = WKT if phase == "k" else WQT
                        for d in range(NDC):
                            nc.sync.dma_start(
                                w_sb[:, d, :], WT[d * 128:(d + 1) * 128, :])
                        for ec in range(NEC):
                            ps_row = psA.tile([128, S], FP32, tag="psrow")
                            for d in range(NDC):
                                for kb in range(NKB):
                                    nc.tensor.matmul(
                                        ps_row[:, kb * 512:(kb + 1) * 512],
                                        w_sb[:, d, ec * 128:(ec + 1) * 128],
                                        xt[:, d, kb * 512:(kb + 1) * 512],
                                        start=(d == 0),
                                        stop=(d == NDC - 1),
                                    )
                            if phase == "k":
                                nc.scalar.activation(
                                    out=kt[:, ec, :], in_=ps_row, func=Copy,
                                    bias=bk_sb[:, ec:ec + 1] if has_bk else 0.0,
                                )
                            else:
                                qsb = qsp.tile([128, S], FP32R, tag="qsb")
                                nc.scalar.activation(
                                    out=qsb, in_=ps_row, func=Copy,
                                    bias=bq_sb[:, ec:ec + 1] if has_bq else 0.0,
                                )
                                nc.sync.dma_start(
                                    qt_dram[:, ec].rearrange("qc p q -> p qc q"),
                                    qsb.rearrange("p (qc q) -> p qc q", q=128),
                                )

                # ---- Phase 3: V with x^T streamed as stationary tiles ----
                # vp stays open through phase B (PV reads v); inner pools close first
                with tc.tile_pool(name="vp", bufs=1) as vp:
                    vq = [vp.tile([128, 4, D], FP32R, name=f"vq{i}_{rep_i}", tag=f"vq{i}")
                          for i in range(4)]  # V quarters: [key%128, kc%4, e]
                    with (
                        tc.tile_pool(name="wp2", bufs=1) as wp2,
                        tc.tile_pool(name="xsp", bufs=3) as xsp,
                        tc.tile_pool(name="psV", bufs=2, space="PSUM") as psV,
                    ):
                        wv_sb = wp2.tile([128, NDC, D], FP32R)
                        for d in range(NDC):
                            nc.sync.dma_start(
                                wv_sb[:, d, :], WVT[d * 128:(d + 1) * 128, :])
                        for kc in range(NQC):
                            xts = xsp.tile([128, NDC, 128], FP32R, tag="xts")
                            nc.sync.dma_start(
                                xts,
                                XT[:, kc * 128:(kc + 1) * 128].rearrange(
                                    "(c p) k -> p c k", p=128
                                ),
                            )
                            ps_v = psV.tile([128, D], FP32, tag="psv")
                            for d in range(NDC):
                                for eb in range(2):
                                    nc.tensor.matmul(
                                        ps_v[:, eb * 512:(eb + 1) * 512],
                                        xts[:, d, :],
                                        wv_sb[:, d, eb * 512:(eb + 1) * 512],
                                        start=(d == 0),
                                        stop=(d == NDC - 1),
                                    )
                            nc.scalar.activation(
                                out=vq[kc // 4][:, kc % 4, :], in_=ps_v, func=Copy)

                    # ---- Phase B: attention per query chunk ----
                    with (
                        tc.tile_pool(name="qtbp", bufs=3) as qtbp,
                        tc.tile_pool(name="esp", bufs=2) as esp,
                        tc.tile_pool(name="ptp", bufs=2) as ptp,
                        tc.tile_pool(name="outp", bufs=2) as outp,
                        tc.tile_pool(name="smallp", bufs=3) as smallp,
                        tc.tile_pool(name="psS", bufs=4, space="PSUM") as psS,
                        tc.tile_pool(name="psT", bufs=2, space="PSUM") as psT,
                        tc.tile_pool(name="psO", bufs=1, space="PSUM") as psO,
                    ):
                        for qc in range(NQC):
                            qt_sb = qtbp.tile([128, NEC, 128], FP32R, tag="qt")
                            nc.sync.dma_start(
                                qt_sb, qt_dram[qc].rearrange("ec p q -> p ec q")
                            )

                            # scores: 4 psum banks [q=128, key=512 each]
                            ps_s = [psS.tile([128, 512], FP32, tag="pss", name=f"pss_{rep_i}_{qc}_{i}") for i in range(NKB)]
                            for ec in range(NEC):
                                for kb in range(NKB):
                                    nc.tensor.matmul(
                                        ps_s[kb],
                                        qt_sb[:, ec, :],
                                        kt[:, ec, kb * 512:(kb + 1) * 512],
                                        start=(ec == 0),
                                        stop=False,
                                    )
                            for kb in range(NKB):
                                nc.tensor.matmul(
                                    ps_s[kb],
                                    ones,
                                    maskb[:, kb * 512:(kb + 1) * 512],
                                    start=False,
                                    stop=True,
                                )

                            es = esp.tile([128, S], FP32R, tag="es")
                            pden = smallp.tile([128, NKB], FP32, tag="pden")
                            for kb in range(NKB):
                                nc.scalar.activation(
                                    out=es[:, kb * 512:(kb + 1) * 512],
                                    in_=ps_s[kb],
                                    func=Exp,
                                    scale=float(SCALE),
                                    accum_out=pden[:, kb:kb + 1],
                                )
                            recip = smallp.tile([128, 1], FP32, tag="recip")
                            den = smallp.tile([128, 1], FP32, tag="den")
                            nc.vector.reduce_sum(den, pden, axis=mybir.AxisListType.X)
                            nc.vector.reciprocal(recip, den)

                            # P^T via PE transpose, 4 per psum bank
                            pt_g = [ptp.tile([128, 4, 128], FP32R,
                                             name=f"ptg{g}_{rep_i}_{qc}", tag=f"ptg{g}")
                                    for g in range(4)]
                            for g in range(4):
                                pt_ps = psT.tile([128, 4, 128], FP32R, tag="ptps")
                                for j in range(4):
                                    kc = 4 * g + j
                                    nc.tensor.transpose(
                                        pt_ps[:, j, :], es[:, kc * 128:(kc + 1) * 128], ident
                                    )
                                nc.vector.tensor_copy(pt_g[g], pt_ps)

                            # PV: out[q, e] accumulated over 16 key chunks
                            ps_o = psO.tile([128, D], FP32, tag="pso")
                            for kc in range(NQC):
                                for eb in range(2):
                                    nc.tensor.matmul(
                                        ps_o[:, eb * 512:(eb + 1) * 512],
                                        pt_g[kc // 4][:, kc % 4, :],
                                        vq[kc // 4][:, kc % 4, eb * 512:(eb + 1) * 512],
                                        start=(kc == 0),
                                        stop=(kc == NQC - 1),
                                    )
                            osb = outp.tile([128, D], FP32, tag="osb")
                            nc.vector.tensor_scalar_mul(osb, ps_o, recip)
                            nc.sync.dma_start(OUT[qc * 128:(qc + 1) * 128, :], osb)

    nc.compile()
    return nc


_NC_CACHE = {}


def _build_in_maps(inputs):
    x = np.asarray(inputs["x"], dtype=np.float32)
    mask = np.asarray(inputs["mask"])
    Wq = np.asarray(inputs["Wq"], dtype=np.float32)
    Wk = np.asarray(inputs["Wk"], dtype=np.float32)
    bq = np.asarray(inputs["bq"], dtype=np.float32)
    bk = np.asarray(inputs["bk"], dtype=np.float32)
    Wv = np.asarray(inputs["Wv"], dtype=np.float32)

    WQT = np.ascontiguousarray(Wq.T)
    WKT = np.ascontiguousarray(Wk.T)
    WVT = np.ascontiguousarray(Wv.T)
    ones = np.ones((1, 128), dtype=np.float32)
    idn = np.eye(128, dtype=np.float32)
    bq_r = np.ascontiguousarray(bq.reshape(NEC, 128).T)
    bk_r = np.ascontiguousarray(bk.reshape(NEC, 128).T)

    in_maps = []
    for b in range(x.shape[0]):
        xT = np.ascontiguousarray(x[b].T)
        mb = np.where(mask[b] != 0, 0.0, -1.0e6).astype(np.float32).reshape(1, S)
        in_maps.append({
            "XT": xT, "WQT": WQT, "WKT": WKT, "WVT": WVT,
            "MB": mb, "ONES": ones, "IDN": idn, "BQ": bq_r, "BK": bk_r,
        })
    return in_maps


def kernel(x, mask, Wq, bq, Wk, bk, Wv, bv):
    x = np.asarray(x, dtype=np.float32)
    bq = np.asarray(bq, dtype=np.float32)
    bk = np.asarray(bk, dtype=np.float32)
    bv = np.asarray(bv, dtype=np.float32)

    B = x.shape[0]
    assert x.shape == (B, S, D) and B == NCORES

    has_bq = bool(np.any(bq != 0.0))
    has_bk = bool(np.any(bk != 0.0))
    key = (has_bq, has_bk)
    if key not in _NC_CACHE:
        _NC_CACHE[key] = build_nc(has_bq, has_bk)
    nc = _NC_CACHE[key]

    in_maps = _build_in_maps({
        "x": x, "mask": mask, "Wq": Wq, "Wk": Wk, "Wv": Wv, "bq": bq, "bk": bk,
    })

    res = run_bass_kernel_spmd(nc, in_maps, core_ids=list(range(NCORES)))
    out = np.stack([res.results[b]["OUT"] for b in range(B)], axis=0)
    if np.any(bv != 0.0):
        out = out + bv[None, None, :]
    return out.astype(np.float32)


BF16 = mybir.dt.bfloat16


def build_nc_bf16(has_bq: bool, has_bk: bool, repeat: int = 1):
    """bf16 variant: all matmul operands bf16, fp32 PSUM accumulation.

    Everything fits in SBUF (no DRAM scratch): Q^T/K^T [e,s] and V [key,e]
    all resident in bf16.
    """
    nc = bacc.Bacc("TRN2", target_bir_lowering=False)

    XT = nc.dram_tensor("XT", [D, S], BF16, kind="ExternalInput")
    WQT = nc.dram_tensor("WQT", [D, D], BF16, kind="ExternalInput")
    WKT = nc.dram_tensor("WKT", [D, D], BF16, kind="ExternalInput")
    WVT = nc.dram_tensor("WVT", [D, D], BF16, kind="ExternalInput")
    MB = nc.dram_tensor("MB", [1, S], BF16, kind="ExternalInput")
    ONES = nc.dram_tensor("ONES", [1, 128], BF16, kind="ExternalInput")
    IDN = nc.dram_tensor("IDN", [128, 128], BF16, kind="ExternalInput")
    BQ = nc.dram_tensor("BQ", [128, NEC], FP32, kind="ExternalInput")
    BK = nc.dram_tensor("BK", [128, NEC], FP32, kind="ExternalInput")
    OUT = nc.dram_tensor("OUT", [S, D], FP32, kind="ExternalOutput")

    Copy = mybir.ActivationFunctionType.Copy
    Exp = mybir.ActivationFunctionType.Exp

    with tile.TileContext(nc) as tc:
        with (
            tc.tile_pool(name="const", bufs=1) as constp,
            tc.tile_pool(name="resp", bufs=1) as resp,
        ):
            ident = constp.tile([128, 128], BF16)
            ones = constp.tile([1, 128], BF16)
            maskb = constp.tile([1, S], BF16)
            nc.sync.dma_start(ident, IDN[:, :])
            nc.sync.dma_start(ones, ONES[:, :])
            nc.sync.dma_start(maskb, MB[:, :])
            bq_sb = constp.tile([128, NEC], FP32)
            bk_sb = constp.tile([128, NEC], FP32)
            if has_bq:
                nc.sync.dma_start(bq_sb, BQ[:, :])
            if has_bk:
                nc.sync.dma_start(bk_sb, BK[:, :])

            # whole-kernel residents (bf16): Q^T, K^T [e%128, e//128, s]; V [k%128, k//128, e]
            qt = resp.tile([128, NEC, S], BF16)
            kt = resp.tile([128, NEC, S], BF16)

            for rep_i in range(repeat):
                with tc.tile_pool(name="vp", bufs=1) as vp:
                    vq = [vp.tile([128, 4, D], BF16, name=f"bvq{i}_{rep_i}", tag=f"vq{i}")
                          for i in range(4)]

                    # ---- Phase A: all three projections, x^T + W resident ----
                    with (
                        tc.tile_pool(name="xtp", bufs=1) as xtp,
                        tc.tile_pool(name="wp", bufs=1) as wp,
                        tc.tile_pool(name="psA", bufs=4, space="PSUM") as psA,
                        tc.tile_pool(name="psV", bufs=2, space="PSUM") as psV,
                    ):
                        xt = xtp.tile([128, NDC, S], BF16)
                        for d in range(NDC):
                            nc.sync.dma_start(
                                xt[:, d, :], XT[d * 128:(d + 1) * 128, :])

                        for phase in ("k", "q"):
                            w_sb = wp.tile([128, NDC, D], BF16, tag="w",
                                           name=f"w{phase}_{rep_i}")
                            WT = WKT if phase == "k" else WQT
                            for d in range(NDC):
                                nc.sync.dma_start(
                                    w_sb[:, d, :], WT[d * 128:(d + 1) * 128, :])
                            dst = kt if phase == "k" else qt
                            bias_sb = bk_sb if phase == "k" else bq_sb
                            has_b = has_bk if phase == "k" else has_bq
                            for ec in range(NEC):
                                ps_row = [psA.tile([128, 512], FP32, tag="psrA",
                                                   name=f"psr{phase}{ec}{i}_{rep_i}")
                                          for i in range(NKB)]
                                for d in range(NDC):
                                    for kb in range(NKB):
                                        nc.tensor.matmul(
                                            ps_row[kb],
                                            w_sb[:, d, ec * 128:(ec + 1) * 128],
                                            xt[:, d, kb * 512:(kb + 1) * 512],
                                            start=(d == 0),
                                            stop=(d == NDC - 1),
                                        )
                                for kb in range(NKB):
                                    nc.scalar.activation(
                                        out=dst[:, ec, kb * 512:(kb + 1) * 512],
                                        in_=ps_row[kb], func=Copy,
                                        bias=bias_sb[:, ec:ec + 1] if has_b else 0.0,
                                    )

                        # V: x^T tiles as stationary (resident!), Wv moving
                        wv_sb = wp.tile([128, NDC, D], BF16, tag="w",
                                        name=f"wv_{rep_i}")
                        for d in range(NDC):
                            nc.sync.dma_start(
                                wv_sb[:, d, :], WVT[d * 128:(d + 1) * 128, :])
                        for kc in range(NQC):
                            ps_v = psV.tile([128, D], FP32, tag="psv")
                            for d in range(NDC):
                                for eb in range(2):
                                    nc.tensor.matmul(
                                        ps_v[:, eb * 512:(eb + 1) * 512],
                                        xt[:, d, kc * 128:(kc + 1) * 128],
                                        wv_sb[:, d, eb * 512:(eb + 1) * 512],
                                        start=(d == 0),
                                        stop=(d == NDC - 1),
                                    )
                            nc.scalar.activation(
                                out=vq[kc // 4][:, kc % 4, :], in_=ps_v, func=Copy)

                    # ---- Phase B ----
                    with (
                        tc.tile_pool(name="esp", bufs=2) as esp,
                        tc.tile_pool(name="ptp", bufs=2) as ptp,
                        tc.tile_pool(name="outp", bufs=2) as outp,
                        tc.tile_pool(name="smallp", bufs=3) as smallp,
                        tc.tile_pool(name="psS", bufs=4, space="PSUM") as psS,
                        tc.tile_pool(name="psT", bufs=2, space="PSUM") as psT,
                        tc.tile_pool(name="psO", bufs=1, space="PSUM") as psO,
                    ):
                        for qc in range(NQC):
                            ps_s = [psS.tile([128, 512], FP32, tag="pss",
                                             name=f"bss_{rep_i}_{qc}_{i}")
                                    for i in range(NKB)]
                            for ec in range(NEC):
                                for kb in range(NKB):
                                    nc.tensor.matmul(
                                        ps_s[kb],
                                        qt[:, ec, qc * 128:(qc + 1) * 128],
                                        kt[:, ec, kb * 512:(kb + 1) * 512],
                                        start=(ec == 0),
                                        stop=False,
                                    )
                            for kb in range(NKB):
                                nc.tensor.matmul(
                                    ps_s[kb],
                                    ones,
                                    maskb[:, kb * 512:(kb + 1) * 512],
                                    start=False,
                                    stop=True,
                                )

                            es = esp.tile([128, S], BF16, tag="es")
                            pden = smallp.tile([128, NKB], FP32, tag="pden")
                            for kb in range(NKB):
                                nc.scalar.activation(
                                    out=es[:, kb * 512:(kb + 1) * 512],
                                    in_=ps_s[kb],
                                    func=Exp,
                                    scale=float(SCALE),
                                    accum_out=pden[:, kb:kb + 1],
                                )
                            recip = smallp.tile([128, 1], FP32, tag="recip")
                            den = smallp.tile([128, 1], FP32, tag="den")
                            nc.vector.reduce_sum(den, pden, axis=mybir.AxisListType.X)
                            nc.vector.reciprocal(recip, den)

                            pt_g = [ptp.tile([128, 4, 128], BF16,
                                             name=f"bptg{g}_{rep_i}_{qc}", tag=f"ptg{g}")
                                    for g in range(4)]
                            for g in range(4):
                                pt_ps = psT.tile([128, 4, 128], BF16, tag="ptps")
                                for j in range(4):
                                    kc = 4 * g + j
                                    nc.tensor.transpose(
                                        pt_ps[:, j, :],
                                        es[:, kc * 128:(kc + 1) * 128], ident)
                                nc.vector.tensor_copy(pt_g[g], pt_ps)

                            ps_o = psO.tile([128, D], FP32, tag="pso")
                            for kc in range(NQC):
                                for eb in range(2):
                                    nc.tensor.matmul(
                                        ps_o[:, eb * 512:(eb + 1) * 512],
                                        pt_g[kc // 4][:, kc % 4, :],
                                        vq[kc // 4][:, kc % 4, eb * 512:(eb + 1) * 512],
                                        start=(kc == 0),
                                        stop=(kc == NQC - 1),
                                    )
                            osb = outp.tile([128, D], FP32, tag="osb")
                            nc.vector.tensor_scalar_mul(osb, ps_o, recip)
                            nc.sync.dma_start(OUT[qc * 128:(qc + 1) * 128, :], osb)

    nc.compile()
    return nc


def _build_in_maps_bf16(inputs):
    import ml_dtypes
    bf = ml_dtypes.bfloat16
    x = np.asarray(inputs["x"], dtype=np.float32)
    mask = np.asarray(inputs["mask"])
    Wq = np.asarray(inputs["Wq"], dtype=np.float32)
    Wk = np.asarray(inputs["Wk"], dtype=np.float32)
    bq = np.asarray(inputs["bq"], dtype=np.float32)
    bk = np.asarray(inputs["bk"], dtype=np.float32)
    Wv = np.asarray(inputs["Wv"], dtype=np.float32)

    WQT = np.ascontiguousarray(Wq.T).astype(bf)
    WKT = np.ascontiguousarray(Wk.T).astype(bf)
    WVT = np.ascontiguousarray(Wv.T).astype(bf)
    ones = np.ones((1, 128), dtype=bf)
    idn = np.eye(128, dtype=np.float32).astype(bf)
    bq_r = np.ascontiguousarray(bq.reshape(NEC, 128).T)
    bk_r = np.ascontiguousarray(bk.reshape(NEC, 128).T)

    in_maps = []
    for b in range(x.shape[0]):
        xT = np.ascontiguousarray(x[b].T).astype(bf)
        mb = np.where(mask[b] != 0, 0.0, -1.0e6).astype(bf).reshape(1, S)
        in_maps.append({
            "XT": xT, "WQT": WQT, "WKT": WKT, "WVT": WVT,
            "MB": mb, "ONES": ones, "IDN": idn, "BQ": bq_r, "BK": bk_r,
        })
    return in_maps



# revision 2
# speedup vs baseline: 1.0597x; 1.0597x over previous
"""Single-head attention (B=8, S=2048, D=1024) on 8 TRN2 NeuronCores.

Data-parallel over batch: core b handles batch element b entirely.

Key optimizations over the naive dataflow:
  1. Mask sparsity: keys with mask==0 contribute exactly nothing to the
     output (exp(-1e9) == 0 in fp32).  The host gathers the unmasked keys
     (~1024 of 2048) and pads to SK (multiple of 128).  K/V projections,
     scores and PV all shrink ~2x.  Mathematically exact.
  2. Transposed scores: S^T[key, q] = (K^T)^T @ Q^T is computed directly
     with key on the PSUM partition dim.  exp(S^T) then directly yields
     P^T tiles for the PV matmul -- no PE transposes at all -- and the
     padding mask is applied as a per-partition bias on the Exp
     activation -- no mask matmuls.
  3. Denominator: V gets an appended ones column; the PV matmul's extra
     output column accumulates sum_k e_k = softmax denominator.
  4. All matmul operands bf16 (1 cycle/row on PE, same rate as fp32r,
     but halves SBUF/DMA so everything stays resident; rel err ~8e-3,
     well inside the 2e-2 gate).

Softmax shift-invariance: reference subtracts rowmax; we subtract
nothing (scores are O(10); exp in fp32 is safe) -- identical result.
bq/bk support: ones-row trick (bias row matmul folded into the psum
accumulation); bv added on host (softmax rows sum to 1).
"""

import sys

sys.path.insert(0, "/opt/trn_rl_repo")

import numpy as np
import ml_dtypes

import concourse.bacc as bacc
import concourse.tile as tile
from concourse import mybir
from concourse.bass_utils import run_bass_kernel_spmd

BF16 = mybir.dt.bfloat16
FP32 = mybir.dt.float32

S = 2048
D = 1024
NCORES = 8
SK_DEFAULT = 1152   # padded unmasked-key count (counts ~1024 +- 35)
NEC = D // 128      # 8 e-chunks
NDC = D // 128      # 8 d-chunks (contraction)
NQC = S // 128      # 16 query chunks
SCALE = 1.0 / np.sqrt(np.float32(D))
PAD_BIAS = -50.0    # exp(scale*0 + PAD_BIAS) == 2e-22: kills padding slots


def _nblocks(n, b=512):
    """Split n into blocks of at most b."""
    out = []
    o = 0
    while o < n:
        out.append((o, min(b, n - o)))
        o += b
    return out


def build_nc(has_bq: bool, has_bk: bool, repeat: int = 1, sk: int = SK_DEFAULT):
    assert sk % 128 == 0
    nkc = sk // 128

    nc = bacc.Bacc("TRN2", target_bir_lowering=False)

    XT = nc.dram_tensor("XT", [D, S], BF16, kind="ExternalInput")
    XGT = nc.dram_tensor("XGT", [D, sk], BF16, kind="ExternalInput")
    WQT = nc.dram_tensor("WQT", [D, D], BF16, kind="ExternalInput")
    WKT = nc.dram_tensor("WKT", [D, D], BF16, kind="ExternalInput")
    WVT = nc.dram_tensor("WVT", [D, D], BF16, kind="ExternalInput")
    MB = nc.dram_tensor("MB", [128, nkc], FP32, kind="ExternalInput")
    BQ = nc.dram_tensor("BQ", [1, D], BF16, kind="ExternalInput")
    BK = nc.dram_tensor("BK", [1, D], BF16, kind="ExternalInput")
    OUT = nc.dram_tensor("OUT", [S, D], FP32, kind="ExternalOutput")

    Copy = mybir.ActivationFunctionType.Copy
    Exp = mybir.ActivationFunctionType.Exp

    with tile.TileContext(nc) as tc:
        with (
            tc.tile_pool(name="const", bufs=1) as constp,
            tc.tile_pool(name="resp", bufs=1) as resp,
        ):
            mb = constp.tile([128, nkc], FP32)
            nc.sync.dma_start(mb, MB[:, :])
            if has_bq or has_bk:
                ones_row = constp.tile([1, S], BF16)
                nc.vector.memset(ones_row, 1.0)
                bq_sb = constp.tile([1, D], BF16)
                bk_sb = constp.tile([1, D], BF16)
                if has_bq:
                    nc.sync.dma_start(bq_sb, BQ[:, :])
                if has_bk:
                    nc.sync.dma_start(bk_sb, BK[:, :])

            # whole-kernel residents (bf16):
            #   Q^T [e%128, e//128, q]; K^T [e%128, e//128, kg]
            #   V   [kg%128, kg//128, e(+1 ones col)]
            qt = resp.tile([128, NEC, S], BF16)
            kt = resp.tile([128, NEC, sk], BF16)
            v = resp.tile([128, nkc, D + 1], BF16)

            for rep_i in range(repeat):
                # ---- Phase A: projections (x^T, x_g^T resident) ----
                with (
                    tc.tile_pool(name="xtp", bufs=1) as xtp,
                    tc.tile_pool(name="wp", bufs=2) as wp,
                ):
                    xt = xtp.tile([128, NDC, S], BF16, name=f"xt_{rep_i}")
                    for d in range(NDC):
                        nc.sync.dma_start(xt[:, d, :], XT[d * 128:(d + 1) * 128, :])
                    xg = xtp.tile([128, NDC, sk], BF16, name=f"xg_{rep_i}")
                    for d in range(NDC):
                        nc.sync.dma_start(xg[:, d, :], XGT[d * 128:(d + 1) * 128, :])

                    # Q^T and K^T: W chunk stationary, x^T moving
                    for phase in ("q", "k"):
                        w_sb = wp.tile([128, NDC, D], BF16, tag="w",
                                       name=f"w{phase}_{rep_i}")
                        WT = WQT if phase == "q" else WKT
                        for d in range(NDC):
                            nc.sync.dma_start(
                                w_sb[:, d, :], WT[d * 128:(d + 1) * 128, :])
                        dst = qt if phase == "q" else kt
                        rhs = xt if phase == "q" else xg
                        blocks = _nblocks(S if phase == "q" else sk)
                        has_b = has_bq if phase == "q" else has_bk
                        b_sb = None
                        if has_b:
                            b_sb = bq_sb if phase == "q" else bk_sb
                        with tc.tile_pool(name=f"psP{phase}", bufs=2,
                                          space="PSUM") as psP:
                            for ec in range(NEC):
                                ps = psP.tile([128, rhs.shape[-1]], FP32,
                                              tag="ps", name=f"ps{phase}{ec}_{rep_i}")
                                for d in range(NDC):
                                    for (o, n) in blocks:
                                        nc.tensor.matmul(
                                            ps[:, o:o + n],
                                            w_sb[:, d, ec * 128:(ec + 1) * 128],
                                            rhs[:, d, o:o + n],
                                            start=(d == 0),
                                            stop=(d == NDC - 1 and not has_b),
                                        )
                                if has_b:
                                    for (o, n) in blocks:
                                        nc.tensor.matmul(
                                            ps[:, o:o + n],
                                            b_sb[0:1, ec * 128:(ec + 1) * 128],
                                            ones_row[0:1, o:o + n],
                                            start=False,
                                            stop=True,
                                        )
                                nc.scalar.activation(
                                    out=dst[:, ec, :], in_=ps, func=Copy)

                    # V: gathered x^T chunk stationary, W_v moving
                    wv_sb = wp.tile([128, NDC, D], BF16, tag="w", name=f"wv_{rep_i}")
                    for d in range(NDC):
                        nc.sync.dma_start(
                            wv_sb[:, d, :], WVT[d * 128:(d + 1) * 128, :])
                    with tc.tile_pool(name="psV", bufs=3, space="PSUM") as psV:
                        for kc in range(nkc):
                            ps_v = psV.tile([128, D], FP32, tag="psv",
                                            name=f"psv{kc}_{rep_i}")
                            for d in range(NDC):
                                for eb in range(2):
                                    nc.tensor.matmul(
                                        ps_v[:, eb * 512:(eb + 1) * 512],
                                        xg[:, d, kc * 128:(kc + 1) * 128],
                                        wv_sb[:, d, eb * 512:(eb + 1) * 512],
                                        start=(d == 0),
                                        stop=(d == NDC - 1),
                                    )
                            nc.scalar.activation(
                                out=v[:, kc, 0:D], in_=ps_v, func=Copy)
                    nc.vector.memset(v[:, :, D:D + 1], 1.0)

                # ---- Phase B: scores^T -> exp -> E^T resident ----
                with tc.tile_pool(name="etp", bufs=1) as etp:
                    et = etp.tile([128, nkc, S], BF16, name=f"et_{rep_i}")
                    with tc.tile_pool(name="psS", bufs=2, space="PSUM") as psS:
                        for kc in range(nkc):
                            pss = psS.tile([128, S], FP32, tag="pss",
                                           name=f"pss{kc}_{rep_i}")
                            for ec in range(NEC):
                                for qb in range(S // 512):
                                    nc.tensor.matmul(
                                        pss[:, qb * 512:(qb + 1) * 512],
                                        kt[:, ec, kc * 128:(kc + 1) * 128],
                                        qt[:, ec, qb * 512:(qb + 1) * 512],
                                        start=(ec == 0),
                                        stop=(ec == NEC - 1),
                                    )
                            nc.scalar.activation(
                                out=et[:, kc, :], in_=pss, func=Exp,
                                scale=float(SCALE), bias=mb[:, kc:kc + 1],
                            )

                    # ---- PV: out[q, e] + denominator via ones column ----
                    with (
                        tc.tile_pool(name="outp", bufs=2) as outp,
                        tc.tile_pool(name="smallp", bufs=3) as smallp,
                        tc.tile_pool(name="psO", bufs=2, space="PSUM") as psO,
                        tc.tile_pool(name="psD", bufs=2, space="PSUM") as psD,
                    ):
                        for qc in range(NQC):
                            ps_o = psO.tile([128, D], FP32, tag="pso",
                                            name=f"pso{qc}_{rep_i}")
                            ps_d = psD.tile([128, 1], FP32, tag="psd",
                                            name=f"psd{qc}_{rep_i}")
                            for kc in range(nkc):
                                lhsT = et[:, kc, qc * 128:(qc + 1) * 128]
                                for eb in range(2):
                                    nc.tensor.matmul(
                                        ps_o[:, eb * 512:(eb + 1) * 512],
                                        lhsT,
                                        v[:, kc, eb * 512:(eb + 1) * 512],
                                        start=(kc == 0),
                                        stop=(kc == nkc - 1),
                                    )
                                nc.tensor.matmul(
                                    ps_d,
                                    lhsT,
                                    v[:, kc, D:D + 1],
                                    start=(kc == 0),
                                    stop=(kc == nkc - 1),
                                )
                            recip = smallp.tile([128, 1], FP32, tag="recip",
                                                name=f"rc{qc}_{rep_i}")
                            nc.vector.reciprocal(recip, ps_d)
                            osb = outp.tile([128, D], FP32, tag="osb",
                                            name=f"osb{qc}_{rep_i}")
                            nc.vector.tensor_scalar_mul(osb, ps_o, recip)
                            nc.sync.dma_start(OUT[qc * 128:(qc + 1) * 128, :], osb)

    nc.compile()
    return nc


_NC_CACHE = {}


def _pick_sk(mask):
    """Smallest supported padded key count covering every batch's count."""
    counts = (np.asarray(mask) != 0).sum(axis=1)
    mx = int(counts.max())
    sk = max(SK_DEFAULT, ((mx + 127) // 128) * 128)
    return min(sk, S), counts


def _build_in_maps(inputs, sk=None):
    bf = ml_dtypes.bfloat16
    x = np.asarray(inputs["x"], dtype=np.float32)
    mask = np.asarray(inputs["mask"])
    Wq = np.asarray(inputs["Wq"], dtype=np.float32)
    Wk = np.asarray(inputs["Wk"], dtype=np.float32)
    Wv = np.asarray(inputs["Wv"], dtype=np.float32)
    bq = np.asarray(inputs.get("bq", np.zeros(D)), dtype=np.float32)
    bk = np.asarray(inputs.get("bk", np.zeros(D)), dtype=np.float32)
    if sk is None:
        sk, _ = _pick_sk(mask)
    nkc = sk // 128

    WQT = np.ascontiguousarray(Wq.T).astype(bf)
    WKT = np.ascontiguousarray(Wk.T).astype(bf)
    WVT = np.ascontiguousarray(Wv.T).astype(bf)
    bq_r = bq.reshape(1, D).astype(bf)
    bk_r = bk.reshape(1, D).astype(bf)

    in_maps = []
    for b in range(x.shape[0]):
        idx = np.nonzero(mask[b])[0]
        c = len(idx)
        assert c <= sk
        xg = np.zeros((sk, D), np.float32)
        xg[:c] = x[b][idx]
        mb = np.zeros(sk, np.float32)
        mb[c:] = PAD_BIAS
        in_maps.append({
            "XT": np.ascontiguousarray(x[b].T).astype(bf),
            "XGT": np.ascontiguousarray(xg.T).astype(bf),
            "WQT": WQT, "WKT": WKT, "WVT": WVT,
            "MB": np.ascontiguousarray(mb.reshape(nkc, 128).T),
            "BQ": bq_r, "BK": bk_r,
        })
    return in_maps


def _cpu_reference_batch(x_b, mask_b, Wq, bq, Wk, bk, Wv, bv):
    """Exact fp32 fallback for degenerate batches (e.g. all keys masked)."""
    q = x_b @ Wq.T + bq
    k = x_b @ Wk.T + bk
    vv = x_b @ Wv.T + bv
    s = (q @ k.T) / np.sqrt(np.float32(D))
    s = np.where(mask_b[None, :] == 0, np.float32(-1e9), s)
    s = s - s.max(axis=1, keepdims=True)
    e = np.exp(s)
    return (e @ vv) / e.sum(axis=1, keepdims=True)


def kernel(x, mask, Wq, bq, Wk, bk, Wv, bv):
    x = np.asarray(x, dtype=np.float32)
    mask = np.asarray(mask)
    bq = np.asarray(bq, dtype=np.float32)
    bk = np.asarray(bk, dtype=np.float32)
    bv = np.asarray(bv, dtype=np.float32)

    B = x.shape[0]
    assert x.shape == (B, S, D) and B == NCORES

    sk, counts = _pick_sk(mask)

    has_bq = bool(np.any(bq != 0.0))
    has_bk = bool(np.any(bk != 0.0))
    key = (has_bq, has_bk, sk)
    if key not in _NC_CACHE:
        _NC_CACHE[key] = build_nc(has_bq, has_bk, sk=sk)
    nc = _NC_CACHE[key]

    in_maps = _build_in_maps(
        {"x": x, "mask": mask, "Wq": Wq, "Wk": Wk, "Wv": Wv,
         "bq": bq, "bk": bk}, sk=sk)

    res = run_bass_kernel_spmd(nc, in_maps, core_ids=list(range(NCORES)))
    out = np.stack([res.results[b]["OUT"] for b in range(B)], axis=0)
    if np.any(bv != 0.0):
        out = out + bv[None, None, :]
    for b in range(B):
        if counts[b] == 0:
            out[b] = _cpu_reference_batch(
                x[b], mask[b], Wq, bq, Wk, bk, Wv, bv)
    return out.astype(np.float32)
